# revision 1
# baseline (speedup 1.0000x reference)
"""Degraded bicycle rollout kernel for Trainium2 (8 NeuronCores, data-parallel on batch).

Math restructure (validated vs reference in numpy):
  - All control-dependent transcendentals (tanh/sigmoid/tan/arctan) hoisted out
    of the sequential scan and computed in parallel over (b,l,h).
  - The scan recurrence only propagates through `speed`, because
    vx^2+vy^2 == speed2^2 exactly. The exact per-step map
        s2_t = max(s_t + accDT_t, 0);  s_{t+1} = sqrt(s2_t^2 + 1e-6)
    runs as an 80-step serial chain (DVE add, DVE relu*u, ACT sqrt).
  - psi / px / py are per-rollout prefix sums -> hardware tensor_tensor_scan
    (segmented via a multiply-mask reset).
  - ax/ay are shifted differences; channels 8..11 and t=0 are control-only.

Layout per core: 4096 rollouts = 128 partitions x 32; partition p holds
rollouts p*32..p*32+31, all from batch bc = p//2 (so per-batch scales are
per-partition scalars). Free dim is rollout-major: f = n*80 + t.
Output staged channel-interleaved in SBUF, DMA'd in 8 rollout-chunks.
"""

import sys

sys.path.insert(0, "/opt/trn_rl_repo")

import numpy as np

B, L, H = 512, 64, 80
NCORES = 8
BC = B // NCORES          # 64 batches per core
R = BC * L                # 4096 rollouts per core
P = 128
NPT = R // P              # 32 rollouts per partition
F = NPT * H               # 2560 elements per partition (dense tiles)
HP1 = H + 1
CW = 12                   # output channels
SW = HP1 * CW             # 972 staging words per rollout
G = 8                     # output chunks (rollout groups)
NG = NPT // G             # 4 rollouts per partition per chunk
CF = NG * H               # 320 free elems per chunk slice
CHW = NG * SW             # staging words per partition per chunk
DT = 0.1
WB = 2.8
PI = float(np.pi)

_BUILT = None


def _build_kernel():
    import concourse.bass as bass
    import concourse.bacc as bacc
    import concourse.mybir as mybir
    from concourse.tile import TileContext
    from concourse.tile_rust import add_dep_helper

    f32 = mybir.dt.float32
    A = mybir.AluOpType
    AF = mybir.ActivationFunctionType

    nc = bacc.Bacc(None, target_bir_lowering=False)
    ctrl_d = nc.declare_dram_parameter("ctrl", [P, NPT * H * 3], f32, isOutput=False)
    x0_d = nc.declare_dram_parameter("x0p", [P, 12], f32, isOutput=False)
    deg_d = nc.declare_dram_parameter("degp", [P, 5], f32, isOutput=False)
    out_d = nc.declare_dram_parameter("out", [P, NPT * SW], f32, isOutput=True)

    with TileContext(nc) as tc:
        v = nc.vector
        sc = nc.scalar
        gp = nc.gpsimd
        sy = nc.sync

        with tc.tile_pool(name="pers", bufs=1) as pp, \
             tc.tile_pool(name="work", bufs=3) as wp, \
             tc.tile_pool(name="late", bufs=1) as lp, \
             tc.tile_pool(name="stgp", bufs=2) as sp, \
             tc.tile_pool(name="cs", bufs=2) as cp, \
             tc.tile_pool(name="psp", bufs=6, space="PSUM") as ps:

            # ---------- small constants ----------
            x0s = pp.tile([P, 12], f32, tag="x0s")
            degs = pp.tile([P, 5], f32, tag="degs")
            sy.dma_start(out=x0s[:], in_=x0_d[:])
            sy.dma_start(out=degs[:], in_=deg_d[:])

            vecs = pp.tile([P, 16], f32, tag="vecs")
            steer = vecs[:, 0:1]
            brake = vecs[:, 1:2]
            thr = vecs[:, 2:3]
            b65 = vecs[:, 3:4]
            t28 = vecs[:, 4:5]
            lo75 = vecs[:, 5:6]
            f981 = vecs[:, 6:7]
            vx010 = vecs[:, 7:8]
            vy010 = vecs[:, 8:9]
            s0q = vecs[:, 9:10]
            s0v = vecs[:, 10:11]
            tmpa = vecs[:, 11:12]
            tmpb = vecs[:, 12:13]
            epsv = vecs[:, 13:14]
            hpiv = vecs[:, 14:15]
            psi0 = x0s[:, 2:3]
            px0 = x0s[:, 0:1]
            py0 = x0s[:, 1:2]
            vx0 = x0s[:, 3:4]
            vy0 = x0s[:, 4:5]

            v.tensor_scalar(steer, degs[:, 0:1], 0.05, None, A.max)
            v.tensor_scalar(brake, degs[:, 1:2], 0.05, None, A.max)
            v.tensor_scalar(thr, degs[:, 2:3], 0.05, None, A.max)
            v.tensor_scalar(b65, degs[:, 1:2], 0.05, 0.65, A.max, A.mult)
            v.tensor_scalar(t28, degs[:, 2:3], 0.05, 0.28, A.max, A.mult)
            v.tensor_scalar(lo75, degs[:, 4:5], 0.1, -0.75, A.max, A.mult)
            v.tensor_scalar(f981, degs[:, 4:5], 0.1, 9.81, A.max, A.mult)
            v.tensor_scalar(vx010, vx0, 10.0, None, A.mult)
            v.tensor_scalar(vy010, vy0, 10.0, None, A.mult)
            v.tensor_tensor(tmpa, vx0, vx0, A.mult)
            v.tensor_tensor(tmpb, vy0, vy0, A.mult)
            v.tensor_tensor(s0q, tmpa, tmpb, A.add)
            v.memset(epsv, 1e-6)
            v.memset(hpiv, PI / 2)

            ones32 = pp.tile([P, NPT], f32, tag="ones32")
            v.memset(ones32[:], 1.0)

            # per-chunk scan mask: 0 at t==0 of each rollout, 1 elsewhere
            maskc = pp.tile([P, CF], f32, tag="maskc")
            v.memset(maskc[:], 1.0)
            mc3 = maskc[:].rearrange("p (n h) -> p n h", n=NG)
            v.memset(mc3[:, :, 0:1], 0.0)

            # ---------- load controls ----------
            ctrl = wp.tile([P, NPT * H * 3], f32, tag="big", bufs=1)
            CH2 = (NPT // 2) * H * 3
            sy.dma_start(out=ctrl[:, 0:CH2], in_=ctrl_d[:, 0:CH2])
            sy.dma_start(out=ctrl[:, CH2:], in_=ctrl_d[:, CH2:])
            c4 = ctrl[:].rearrange("p (n h c) -> p n h c", n=NPT, h=H)
            u0 = c4[:, :, :, 0]
            u1 = c4[:, :, :, 1]
            u2 = c4[:, :, :, 2]

            # ---------- phase A: control transforms ----------
            HF = F // 2
            HN = NPT // 2
            sg1 = wp.tile([P, F], f32, tag="W2", bufs=1)
            sg13 = sg1[:].rearrange("p (n h) -> p n h", n=NPT)
            i_sg1a = sc.activation(sg13[:, 0:HN, :], u1[:, 0:HN, :], AF.Sigmoid)
            sg2 = wp.tile([P, F], f32, tag="W3", bufs=1)
            sg23 = sg2[:].rearrange("p (n h) -> p n h", n=NPT)
            i_sg2a = sc.activation(sg23[:, 0:HN, :], u2[:, 0:HN, :], AF.Sigmoid)
            th = wp.tile([P, F], f32, tag="W1", bufs=1)
            th3 = th[:].rearrange("p (n h) -> p n h", n=NPT)
            i_tanh = sc.activation(th3, u0, AF.Tanh)
            i_sg1b = sc.activation(sg13[:, HN:, :], u1[:, HN:, :], AF.Sigmoid)
            i_sg2b = sc.activation(sg23[:, HN:, :], u2[:, HN:, :], AF.Sigmoid)

            fb65 = wp.tile([P, F], f32, tag="W1p", bufs=1)
            v.tensor_scalar(fb65[:, 0:HF], sg1[:, 0:HF], b65, None, A.mult)
            t3 = wp.tile([P, F], f32, tag="W2p", bufs=1)
            v.scalar_tensor_tensor(t3[:, 0:HF], sg2[:, 0:HF], t28,
                                   fb65[:, 0:HF], A.mult, A.subtract)
            accDT = wp.tile([P, F], f32, tag="accdt", bufs=1)
            v.tensor_scalar(accDT[:, 0:HF], t3[:, 0:HF], 0.3, lo75,
                            A.min, A.max)
            v.tensor_scalar(fb65[:, HF:], sg1[:, HF:], b65, None, A.mult)
            v.scalar_tensor_tensor(t3[:, HF:], sg2[:, HF:], t28,
                                   fb65[:, HF:], A.mult, A.subtract)
            v.tensor_scalar(accDT[:, HF:], t3[:, HF:], 0.3, lo75,
                            A.min, A.max)

            delta = lp.tile([P, F], f32, tag="delta")
            v.tensor_scalar(delta[:], th[:], steer, None, A.mult)
            dc = wp.tile([P, F], f32, tag="W4", bufs=1)
            v.tensor_scalar(dc[:], delta[:], 0.75, -0.75, A.min, A.max)
            fb = lp.tile([P, F], f32, tag="fb")
            v.tensor_scalar(fb[:], sg1[:], brake, None, A.mult)
            fx = lp.tile([P, F], f32, tag="fx")
            v.tensor_scalar(fx[:], sg2[:], thr, None, A.mult)

            # ---------- phase B: serial speed recurrence (exact) ----------
            i_s0 = sc.activation(s0v, s0q, AF.Sqrt, bias=epsv)
            add_dep_helper(i_s0.ins, i_tanh.ins, reason="act set sig->sqrt")
            add_dep_helper(i_s0.ins, i_sg1b.ins, reason="act set sig->sqrt")
            add_dep_helper(i_s0.ins, i_sg2b.ins, reason="act set sig->sqrt")

            HNP = NPT // 2
            s_curA = pp.tile([P, HNP], f32, tag="s_a0")
            s_nxtA = pp.tile([P, HNP], f32, tag="s_a1")
            u_tA = pp.tile([P, HNP], f32, tag="u_a")
            s_curB = pp.tile([P, HNP], f32, tag="s_b0")
            s_nxtB = pp.tile([P, HNP], f32, tag="s_b1")
            u_tB = pp.tile([P, HNP], f32, tag="u_b")
            v.tensor_scalar(s_curA[:], ones32[:, 0:HNP], s0v, None, A.mult)
            v.tensor_scalar(s_curB[:], ones32[:, 0:HNP], s0v, None, A.mult)

            qA = pp.tile([P, HNP], f32, tag="q_a")
            qB = pp.tile([P, HNP], f32, tag="q_b")
            ufull = wp.tile([P, F], f32, tag="big", bufs=1)
            a3 = accDT[:].rearrange("p (n h) -> p n h", n=NPT)
            u3 = ufull[:].rearrange("p (n h) -> p n h", n=NPT)
            last_sqrt = None
            for t in range(H):
                v.tensor_tensor(u3[:, 0:HNP, t], s_curA[:], a3[:, 0:HNP, t],
                                A.add)
                v.scalar_tensor_tensor(qA[:], u3[:, 0:HNP, t], 0.0,
                                       u3[:, 0:HNP, t], A.max, A.mult)
                i_sqA = sc.activation(s_nxtA[:], qA[:], AF.Sqrt, bias=epsv)
                v.tensor_tensor(u3[:, HNP:, t], s_curB[:], a3[:, HNP:, t],
                                A.add)
                v.scalar_tensor_tensor(qB[:], u3[:, HNP:, t], 0.0,
                                       u3[:, HNP:, t], A.max, A.mult)
                i_sqB = sc.activation(s_nxtB[:], qB[:], AF.Sqrt, bias=epsv)
                if last_sqrt is None:
                    add_dep_helper(i_sqA.ins, i_s0.ins, reason="chain after s0")
                    add_dep_helper(i_sqB.ins, i_s0.ins, reason="chain after s0")
                s_curA, s_nxtA = s_nxtA, s_curA
                s_curB, s_nxtB = s_nxtB, s_curB
                last_sqrt = i_sqB

            # trig passes after the sqrt chain (table-set order)
            sin_d = wp.tile([P, F], f32, tag="W1p", bufs=1)
            i_sind = sc.activation(sin_d[:], dc[:], AF.Sin)
            add_dep_helper(i_sind.ins, last_sqrt.ins,
                           reason="act set order sqrt->trig")
            cos_d = wp.tile([P, F], f32, tag="W2p", bufs=1)
            i_cosd = sc.activation(cos_d[:], dc[:], AF.Sin, bias=hpiv)
            rc = wp.tile([P, F], f32, tag="W3", bufs=1)
            v.reciprocal_approx_fast(rc[:], cos_d[:])
            tan045 = wp.tile([P, F], f32, tag="t045", bufs=1)
            v.scalar_tensor_tensor(tan045[:], sin_d[:], 0.45, rc[:],
                                   A.mult, A.mult)

            t453 = tan045[:].rearrange("p (n h) -> p n h", n=NPT)
            d3 = delta[:].rearrange("p (n h) -> p n h", n=NPT)
            fbb3 = fb[:].rearrange("p (n h) -> p n h", n=NPT)
            fxx3 = fx[:].rearrange("p (n h) -> p n h", n=NPT)

            first_arctan = [None]

            # ---------- phase D/E: per-chunk pipeline + staging + DMA ----------
            for g in range(G):
                n0 = g * NG
                us = u3[:, n0:n0 + NG, :].rearrange("p n h -> p (n h)")
                t45c = t453[:, n0:n0 + NG, :].rearrange("p n h -> p (n h)")

                # per-chunk yaw chain: s2 = relu(u) (exact; no ACT sqrt)
                s2c = cp.tile([P, CF], f32, tag="s2c", bufs=2)
                v.tensor_scalar(s2c[:], us, 0.0, None, A.max)
                mch = cp.tile([P, CF], f32, tag="mch", bufs=1)
                v.tensor_scalar(mch[:], s2c[:], 2.0, None, A.max)
                imc = cp.tile([P, CF], f32, tag="imc", bufs=1)
                v.reciprocal_approx_fast(imc[:], mch[:])
                rawc = cp.tile([P, CF], f32, tag="rawc", bufs=1)
                v.scalar_tensor_tensor(rawc[:], s2c[:], 1.0 / (0.45 * WB),
                                       t45c, A.mult, A.mult)
                clpc = cp.tile([P, CF], f32, tag="clpc", bufs=1)
                v.tensor_scalar(clpc[:], rawc[:], 1.0, -1.0, A.min, A.max)
                ylc = cp.tile([P, CF], f32, tag="ylc", bufs=1)
                v.tensor_scalar(ylc[:], imc[:], f981, None, A.mult)
                yawc = cp.tile([P, CF], f32, tag="yawc", bufs=2)
                v.scalar_tensor_tensor(yawc[:], ylc[:], 0.15, clpc[:],
                                       A.max, A.mult)
                ys = yawc[:]
                ss = s2c[:]

                stg = sp.tile([P, CHW], f32, tag="stg")
                s4 = stg[:].rearrange("p (n t c) -> p n t c", n=NG, t=HP1)

                # t=0 slice: 12 channels = x0 row, broadcast over rollouts
                x0b = x0s[:, None, 0:12].broadcast_to([P, NG, 12])
                v.tensor_scalar(s4[:, :, 0, :], x0b, 1.0, None, A.mult)

                # control-only channels (strided copies into staging)
                gp.tensor_scalar(s4[:, :, 1:, 9], d3[:, n0:n0 + NG, :],
                                 1.0, None, A.mult)
                gp.tensor_scalar(s4[:, :, 1:, 10], fbb3[:, n0:n0 + NG, :],
                                 1.0, None, A.mult)
                gp.tensor_scalar(s4[:, :, 1:, 11], fxx3[:, n0:n0 + NG, :],
                                 1.0, None, A.mult)
                i_bt = sc.activation(s4[:, :, 1:, 8], t453[:, n0:n0 + NG, :],
                                     AF.Arctan)
                if first_arctan[0] is None:
                    first_arctan[0] = i_bt
                    add_dep_helper(i_bt.ins, last_sqrt.ins,
                                   reason="act set order sqrt->trig")
                sc.copy(s4[:, :, 1:, 5],
                        yawc[:].rearrange("p (n h) -> p n h", n=NG))

                # psi = psi0 + 0.1 * segmented-cumsum(yawr)
                Fp = cp.tile([P, CF], f32, tag="Fp", bufs=2)
                v.tensor_tensor_scan(Fp[:], maskc[:], ys, 0.0, A.mult, A.add)
                sc.activation(s4[:, :, 1:, 2],
                              Fp[:].rearrange("p (n h) -> p n h", n=NG),
                              AF.Identity, bias=psi0, scale=DT)

                # arg = psi + beta ; wrap into [-pi, pi] for ACT sin
                argc = ps.tile([P, CF], f32, tag="pst")
                v.tensor_tensor(argc[:].rearrange("p (n h) -> p n h", n=NG),
                                s4[:, :, 1:, 2], s4[:, :, 1:, 8], A.add)
                argw = ps.tile([P, CF], f32, tag="pst")
                v.add_range_wrap(argw[:], argc[:], 0.0, PI, 2 * PI)
                cwv = ps.tile([P, CF], f32, tag="pst")
                v.add_range_wrap(cwv[:], argc[:], PI / 2, PI, 2 * PI)
                sinA = cp.tile([P, CF], f32, tag="sinA")
                sc.activation(sinA[:], argw[:], AF.Sin)
                cosA = cp.tile([P, CF], f32, tag="cosA")
                sc.activation(cosA[:], cwv[:], AF.Sin)

                vx2 = cp.tile([P, CF], f32, tag="vx2")
                v.tensor_tensor(vx2[:], ss, cosA[:], A.mult)
                vy2 = cp.tile([P, CF], f32, tag="vy2")
                v.tensor_tensor(vy2[:], ss, sinA[:], A.mult)

                vx3 = vx2[:].rearrange("p (n h) -> p n h", n=NG)
                vy3 = vy2[:].rearrange("p (n h) -> p n h", n=NG)
                sc.copy(s4[:, :, 1:, 3], vx3)
                sc.copy(s4[:, :, 1:, 4], vy3)

                # px/py via segmented cumsum of vx2/vy2
                Fx = cp.tile([P, CF], f32, tag="Fx", bufs=2)
                v.tensor_tensor_scan(Fx[:], maskc[:], vx2[:], 0.0, A.mult, A.add)
                sc.activation(s4[:, :, 1:, 0],
                              Fx[:].rearrange("p (n h) -> p n h", n=NG),
                              AF.Identity, bias=px0, scale=DT)
                Fy = cp.tile([P, CF], f32, tag="Fy", bufs=2)
                v.tensor_tensor_scan(Fy[:], maskc[:], vy2[:], 0.0, A.mult, A.add)
                sc.activation(s4[:, :, 1:, 1],
                              Fy[:].rearrange("p (n h) -> p n h", n=NG),
                              AF.Identity, bias=py0, scale=DT)

                # ax/ay: shifted diffs (t>=1); t=0 against vx0/vy0
                dxc = cp.tile([P, NG * (H - 1)], f32, tag="ddx", bufs=1)
                d3c = dxc[:].rearrange("p (n h) -> p n h", n=NG)
                gp.tensor_tensor(d3c, vx3[:, :, 1:], vx3[:, :, :H - 1], A.subtract)
                v.tensor_scalar(s4[:, :, 2:, 6], d3c, 10.0, None, A.mult)
                v.tensor_scalar(s4[:, :, 1, 6], vx3[:, :, 0], 10.0, vx010,
                                A.mult, A.subtract)
                dyc = cp.tile([P, NG * (H - 1)], f32, tag="ddy", bufs=1)
                dy3 = dyc[:].rearrange("p (n h) -> p n h", n=NG)
                gp.tensor_tensor(dy3, vy3[:, :, 1:], vy3[:, :, :H - 1], A.subtract)
                v.tensor_scalar(s4[:, :, 2:, 7], dy3, 10.0, None, A.mult)
                v.tensor_scalar(s4[:, :, 1, 7], vy3[:, :, 0], 10.0, vy010,
                                A.mult, A.subtract)

                sy.dma_start(out=out_d[:, g * CHW:(g + 1) * CHW], in_=stg[:])

    nc.compile()
    return nc


def _get_built():
    global _BUILT
    if _BUILT is None:
        _BUILT = _build_kernel()
    return _BUILT


def _run(x0, controls, deg, trace=False):
    from concourse.bass_utils import run_bass_kernel_spmd

    x0 = np.ascontiguousarray(x0, dtype=np.float32)
    controls = np.ascontiguousarray(controls, dtype=np.float32)
    deg = np.ascontiguousarray(deg, dtype=np.float32)

    nc = _get_built()
    in_maps = []
    for c in range(NCORES):
        sl = slice(c * BC, (c + 1) * BC)
        ctrl_c = controls[sl].reshape(R, H * 3).reshape(P, NPT * H * 3)
        x0p = np.repeat(x0[sl], P // BC, axis=0)      # [128, 12]
        degp = np.repeat(deg[sl], P // BC, axis=0)    # [128, 5]
        in_maps.append({
            "ctrl": np.ascontiguousarray(ctrl_c),
            "x0p": np.ascontiguousarray(x0p),
            "degp": np.ascontiguousarray(degp),
        })

    res = run_bass_kernel_spmd(nc, in_maps, list(range(NCORES)), trace=trace)
    outs = []
    for c in range(NCORES):
        o = np.asarray(res.results[c]["out"])
        outs.append(o.reshape(R, HP1, CW).reshape(BC, L, HP1, CW))
    return np.concatenate(outs, axis=0), res


def kernel(x0: np.ndarray, controls: np.ndarray, deg: np.ndarray) -> np.ndarray:
    out, _ = _run(x0, controls, deg)
    return out


if __name__ == "__main__":
    rng = np.random.default_rng(0)
    x0 = rng.standard_normal((B, 12)).astype(np.float32)
    controls = rng.standard_normal((B, L, H, 3)).astype(np.float32)
    deg = rng.random((B, 5)).astype(np.float32)
    out = kernel(x0, controls, deg)
    print("out", out.shape, out.dtype)



# revision 12
# speedup vs baseline: 1.5256x; 1.5256x over previous
"""Degraded bicycle rollout kernel for Trainium2 (8 NeuronCores, data-parallel on batch).

v2 restructure (validated vs reference in numpy, worst rel err ~5e-4 vs 2e-2 tol):
  - Serial 80-step speed recurrence replaced by 3 parallel scans using
        relu-prefix(a)_t = A_t - runmin(min(A,0))_t,  A = s0 + cumsum(a)
    (drops the 1e-6 epsilon inside sqrt; error <= ~1e-3 absolute, non-compounding).
  - Per-partition scalars (deg-derived scales, s0, x0 columns) precomputed on host.
  - 4 chunks x 8 rollouts; channels written strided directly into interleaved
    staging; work balanced across DVE / ACT / GPSIMD under the DMA roofline.
  - 2 activation-table loads total: tanh/sigmoid (phase A), then sin/arctan/identity.

Layout per core: 4096 rollouts = 128 partitions x 32; partition p holds
rollouts p*32..p*32+31, all from batch bc = p//2 (per-batch scalars are
per-partition scalars). Free dim rollout-major: f = n*80 + t.
"""

import sys

sys.path.insert(0, "/opt/trn_rl_repo")

import numpy as np

B, L, H = 512, 64, 80
NCORES = 8
BC = B // NCORES          # 64 batches per core
R = BC * L                # 4096 rollouts per core
P = 128
NPT = R // P              # 32 rollouts per partition
F = NPT * H               # 2560 elements per partition
HP1 = H + 1
CW = 12                   # output channels
SW = HP1 * CW             # 972 staging words per rollout
NCH = 4                   # chunks
NG = NPT // NCH           # 8 rollouts per partition per chunk
CF = NG * H               # 640 free elems per chunk
CHW = NG * SW             # 7776 staging words per partition per chunk
QW = NG * H * 3           # 1920 ctrl words per partition per quarter
DT = 0.1
WB = 2.8
PI = float(np.pi)
BIG = 1e30

_BUILT = None


def _build_kernel():
    import concourse.bass as bass
    import concourse.bacc as bacc
    import concourse.mybir as mybir
    from concourse.tile import TileContext
    from concourse.tile_rust import add_dep_helper

    f32 = mybir.dt.float32
    A = mybir.AluOpType
    AF = mybir.ActivationFunctionType

    nc = bacc.Bacc(None, target_bir_lowering=False)
    ctrl_d = nc.declare_dram_parameter("ctrl", [P, NPT * H * 3], f32, isOutput=False)
    x0_d = nc.declare_dram_parameter("x0p", [P, 12], f32, isOutput=False)
    scl_d = nc.declare_dram_parameter("sclp", [P, 16], f32, isOutput=False)
    out_d = nc.declare_dram_parameter("out", [P, NPT * SW], f32, isOutput=True)

    with TileContext(nc) as tc:
        v = nc.vector
        sc = nc.scalar
        gp = nc.gpsimd
        sy = nc.sync

        with tc.tile_pool(name="pers", bufs=1) as pp, \
             tc.tile_pool(name="wk1", bufs=1) as w1, \
             tc.tile_pool(name="wk2", bufs=2) as w2, \
             tc.tile_pool(name="stgp", bufs=2) as sp, \
             tc.tile_pool(name="psp", bufs=1, space="PSUM") as ps:

            x0s = pp.tile([P, 12], f32, tag="x0s")
            scl = pp.tile([P, 16], f32, tag="scl")
            sy.dma_start(out=x0s[:], in_=x0_d[:])
            sy.dma_start(out=scl[:], in_=scl_d[:])
            steer = scl[:, 0:1]
            brake = scl[:, 1:2]
            thr = scl[:, 2:3]
            b65 = scl[:, 3:4]
            t28 = scl[:, 4:5]
            lo75 = scl[:, 5:6]
            invf981 = scl[:, 6:7]
            s0v = scl[:, 7:8]
            psi0 = scl[:, 8:9]
            px0 = scl[:, 9:10]
            py0 = scl[:, 10:11]
            vx010 = scl[:, 11:12]
            vy010 = scl[:, 12:13]
            hpiv = scl[:, 13:14]

            # scan masks (same pattern for every chunk)
            maskc = pp.tile([P, CF], f32, tag="maskc")
            v.memset(maskc[:], 1.0)
            mc3 = maskc[:].rearrange("p (n h) -> p n h", n=NG)
            v.memset(mc3[:, :, 0:1], 0.0)
            bigm = pp.tile([P, CF], f32, tag="bigm")
            v.memset(bigm[:], 0.0)
            bg3 = bigm[:].rearrange("p (n h) -> p n h", n=NG)
            v.memset(bg3[:, :, 0:1], BIG)

            # persistent control transforms
            th = pp.tile([P, F], f32, tag="th")
            sg1 = pp.tile([P, F], f32, tag="sg1")
            sg2 = pp.tile([P, F], f32, tag="sg2")
            accDT = pp.tile([P, F], f32, tag="accDT")
            th3 = th[:].rearrange("p (n h) -> p n h", n=NPT)
            sg13 = sg1[:].rearrange("p (n h) -> p n h", n=NPT)
            sg23 = sg2[:].rearrange("p (n h) -> p n h", n=NPT)

            ctrl = w1.tile([P, NPT * H * 3], f32, tag="ctrl")
            c4 = ctrl[:].rearrange("p (n h c) -> p n h c", n=NPT, h=H)

            # ---------- phase A: per-quarter input DMA + tanh/sigmoid ----------
            last_phaseA_act = None
            for q in range(NCH):
                n0 = q * NG
                sy.dma_start(out=ctrl[:, q * QW:(q + 1) * QW],
                             in_=ctrl_d[:, q * QW:(q + 1) * QW])
                nsl = slice(n0, n0 + NG)
                sc.activation(th3[:, nsl, :], c4[:, nsl, :, 0], AF.Tanh)
                sc.activation(sg13[:, nsl, :], c4[:, nsl, :, 1], AF.Sigmoid)
                i_sg2 = sc.activation(sg23[:, nsl, :], c4[:, nsl, :, 2], AF.Sigmoid)
                last_phaseA_act = i_sg2
                csl = slice(q * CF, (q + 1) * CF)
                fb65 = w1.tile([P, CF], f32, tag=f"fb65_{q % 2}")
                v.tensor_scalar(fb65[:], sg1[:, csl], b65, None, A.mult)
                t3 = w1.tile([P, CF], f32, tag=f"t3_{q % 2}")
                v.scalar_tensor_tensor(t3[:], sg2[:, csl], t28, fb65[:],
                                       A.mult, A.subtract)
                v.tensor_scalar(accDT[:, csl], t3[:], 0.3, lo75, A.min, A.max)

            # ---------- chunk loop ----------
            first_trig = [None]
            for g in range(NCH):
                n0 = g * NG
                csl = slice(g * CF, (g + 1) * CF)
                nsl = slice(n0, n0 + NG)

                stg = sp.tile([P, CHW], f32, tag="stg")
                s4 = stg[:].rearrange("p (n t c) -> p n t c", n=NG, t=HP1)

                # t=0 slice: full x0 row broadcast over rollouts
                x0b = x0s[:, None, 0:12].broadcast_to([P, NG, 12])
                v.tensor_scalar(s4[:, :, 0, :], x0b, 1.0, None, A.mult)

                # ch9 delta = steer*tanh(u0); dc = clip(delta, +-0.75)
                v.tensor_scalar(s4[:, :, 1:, 9], th3[:, nsl, :], steer, None,
                                A.mult)
                dc = w1.tile([P, CF], f32, tag="dc")
                dc3 = dc[:].rearrange("p (n h) -> p n h", n=NG)
                v.tensor_scalar(dc3, s4[:, :, 1:, 9], 0.75, -0.75, A.min, A.max)

                # trig of steering
                sin_d = w2.tile([P, CF], f32, tag="sin_d")
                i_sin = sc.activation(sin_d[:], dc[:], AF.Sin)
                if first_trig[0] is None:
                    first_trig[0] = i_sin
                    add_dep_helper(i_sin.ins, last_phaseA_act.ins,
                                   reason="act table: sigmoid-set before trig-set")
                cos_d = w2.tile([P, CF], f32, tag="cos_d")
                sc.activation(cos_d[:], dc[:], AF.Sin, bias=hpiv)
                rc = w1.tile([P, CF], f32, tag="rc")
                v.reciprocal_approx_fast(rc[:], cos_d[:])
                tan = w1.tile([P, CF], f32, tag="tan")
                gp.tensor_tensor(tan[:], sin_d[:], rc[:], A.mult)
                tan3 = tan[:].rearrange("p (n h) -> p n h", n=NG)
                sc.activation(s4[:, :, 1:, 8], tan3, AF.Arctan, scale=0.45)

                # speed2 via scans: C=cumsum(accDT); s2 = (C+s0) - runmin(min(C+s0,0))
                C = w1.tile([P, CF], f32, tag="C")
                v.tensor_tensor_scan(C[:], maskc[:], accDT[:, csl], 0.0,
                                     A.mult, A.add)
                Bm = w1.tile([P, CF], f32, tag="w_a")
                v.tensor_scalar(Bm[:], C[:], s0v, 0.0, A.add, A.min)
                mrun = w1.tile([P, CF], f32, tag="mrun")
                v.tensor_tensor_scan(mrun[:], bigm[:], Bm[:], 0.0, A.add, A.min)
                s2 = w2.tile([P, CF], f32, tag="s2")
                v.scalar_tensor_tensor(s2[:], C[:], s0v, mrun[:], A.add,
                                       A.subtract)

                # yaw_rate = clip(s2*tan/WB, +-1) * max(9.81*fric/max(s2,2), 0.15)
                mchi = w1.tile([P, CF], f32, tag="w_b")
                v.tensor_scalar(mchi[:], s2[:], 2.0, invf981, A.max, A.mult)
                imc = w1.tile([P, CF], f32, tag="imc")
                v.reciprocal_approx_fast(imc[:], mchi[:])
                rawc = w1.tile([P, CF], f32, tag="w_a")
                v.scalar_tensor_tensor(rawc[:], s2[:], 1.0 / WB, tan[:],
                                       A.mult, A.mult)
                clpc = w1.tile([P, CF], f32, tag="w_b")
                v.tensor_scalar(clpc[:], rawc[:], 1.0, -1.0, A.min, A.max)
                yawc = w1.tile([P, CF], f32, tag="yawc")
                v.scalar_tensor_tensor(yawc[:], imc[:], 0.15, clpc[:],
                                       A.max, A.mult)
                yaw3 = yawc[:].rearrange("p (n h) -> p n h", n=NG)
                v.tensor_scalar(s4[:, :, 1:, 5], yaw3, 1.0, None, A.mult)

                # psi into ch2
                Fp = w1.tile([P, CF], f32, tag="Fp")
                v.tensor_tensor_scan(Fp[:], maskc[:], yawc[:], 0.0, A.mult, A.add)
                Fp3 = Fp[:].rearrange("p (n h) -> p n h", n=NG)
                sc.activation(s4[:, :, 1:, 2], Fp3, AF.Identity, bias=psi0,
                              scale=DT)

                # arg = psi + beta, wrapped for sin/cos
                argc = w1.tile([P, CF], f32, tag="dc")
                ac3 = argc[:].rearrange("p (n h) -> p n h", n=NG)
                gp.tensor_tensor(ac3, s4[:, :, 1:, 2], s4[:, :, 1:, 8], A.add)
                argw = ps.tile([P, CF], f32, tag="argw")
                v.add_range_wrap(argw[:], argc[:], 0.0, PI, 2 * PI)
                cwv = ps.tile([P, CF], f32, tag="cwv")
                v.add_range_wrap(cwv[:], argc[:], PI / 2, PI, 2 * PI)
                sinA = w1.tile([P, CF], f32, tag="sinA")
                sc.activation(sinA[:], argw[:], AF.Sin)
                cosA = w1.tile([P, CF], f32, tag="cosA")
                sc.activation(cosA[:], cwv[:], AF.Sin)

                # velocities
                vx2 = w2.tile([P, CF], f32, tag="vx2")
                gp.tensor_tensor(vx2[:], s2[:], cosA[:], A.mult)
                vy2 = w2.tile([P, CF], f32, tag="vy2")
                gp.tensor_tensor(vy2[:], s2[:], sinA[:], A.mult)
                vx3 = vx2[:].rearrange("p (n h) -> p n h", n=NG)
                vy3 = vy2[:].rearrange("p (n h) -> p n h", n=NG)
                sc.activation(s4[:, :, 1:, 3], vx3, AF.Identity)
                sc.activation(s4[:, :, 1:, 4], vy3, AF.Identity)

                # px/py into ch0/ch1
                Fx = w1.tile([P, CF], f32, tag="Fx")
                v.tensor_tensor_scan(Fx[:], maskc[:], vx2[:], 0.0, A.mult, A.add)
                Fx3 = Fx[:].rearrange("p (n h) -> p n h", n=NG)
                sc.activation(s4[:, :, 1:, 0], Fx3, AF.Identity, bias=px0,
                              scale=DT)
                Fy = w1.tile([P, CF], f32, tag="Fy")
                v.tensor_tensor_scan(Fy[:], maskc[:], vy2[:], 0.0, A.mult, A.add)
                Fy3 = Fy[:].rearrange("p (n h) -> p n h", n=NG)
                sc.activation(s4[:, :, 1:, 1], Fy3, AF.Identity, bias=py0,
                              scale=DT)

                # ax/ay: diff into staging, then in-place *10 on ACT
                gp.tensor_tensor(s4[:, :, 2:, 6], vx3[:, :, 1:], vx3[:, :, :H - 1],
                                 A.subtract)
                sc.activation(s4[:, :, 2:, 6], s4[:, :, 2:, 6], AF.Identity,
                              scale=10.0)
                v.tensor_scalar(s4[:, :, 1, 6], vx3[:, :, 0], 10.0, vx010,
                                A.mult, A.subtract)
                gp.tensor_tensor(s4[:, :, 2:, 7], vy3[:, :, 1:], vy3[:, :, :H - 1],
                                 A.subtract)
                sc.activation(s4[:, :, 2:, 7], s4[:, :, 2:, 7], AF.Identity,
                              scale=10.0)
                v.tensor_scalar(s4[:, :, 1, 7], vy3[:, :, 0], 10.0, vy010,
                                A.mult, A.subtract)

                # ch10/ch11: brake*sg1, thr*sg2
                gp.tensor_scalar(s4[:, :, 1:, 10], sg13[:, nsl, :], brake, None,
                                 A.mult)
                gp.tensor_scalar(s4[:, :, 1:, 11], sg23[:, nsl, :], thr, None,
                                 A.mult)

                sy.dma_start(out=out_d[:, g * CHW:(g + 1) * CHW], in_=stg[:])

    nc.compile()
    return nc


def _get_built():
    global _BUILT
    if _BUILT is None:
        _BUILT = _build_kernel()
    return _BUILT


def _make_scalars(x0, deg):
    """Per-batch scalar table [B, 16] (host precompute of deg-derived scales)."""
    steer = np.maximum(deg[:, 0], 0.05)
    brake = np.maximum(deg[:, 1], 0.05)
    thr = np.maximum(deg[:, 2], 0.05)
    fric = np.maximum(deg[:, 4], 0.1)
    vx0 = x0[:, 3]
    vy0 = x0[:, 4]
    scl = np.zeros((B, 16), np.float32)
    scl[:, 0] = steer
    scl[:, 1] = brake
    scl[:, 2] = thr
    scl[:, 3] = 0.65 * brake
    scl[:, 4] = 0.28 * thr
    scl[:, 5] = -0.75 * fric
    scl[:, 6] = 1.0 / (9.81 * fric)
    scl[:, 7] = np.sqrt(vx0 * vx0 + vy0 * vy0 + 1e-6)
    scl[:, 8] = x0[:, 2]
    scl[:, 9] = x0[:, 0]
    scl[:, 10] = x0[:, 1]
    scl[:, 11] = 10.0 * vx0
    scl[:, 12] = 10.0 * vy0
    scl[:, 13] = PI / 2
    return scl


def _run(x0, controls, deg, trace=False):
    from concourse.bass_utils import run_bass_kernel_spmd

    x0 = np.ascontiguousarray(x0, dtype=np.float32)
    controls = np.ascontiguousarray(controls, dtype=np.float32)
    deg = np.ascontiguousarray(deg, dtype=np.float32)
    scl = _make_scalars(x0, deg)

    nc = _get_built()
    rep = P // BC
    in_maps = []
    for c in range(NCORES):
        sl = slice(c * BC, (c + 1) * BC)
        ctrl_c = controls[sl].reshape(R, H * 3).reshape(P, NPT * H * 3)
        in_maps.append({
            "ctrl": np.ascontiguousarray(ctrl_c),
            "x0p": np.ascontiguousarray(np.repeat(x0[sl], rep, axis=0)),
            "sclp": np.ascontiguousarray(np.repeat(scl[sl], rep, axis=0)),
        })

    res = run_bass_kernel_spmd(nc, in_maps, list(range(NCORES)), trace=trace)
    outs = []
    for c in range(NCORES):
        o = np.asarray(res.results[c]["out"])
        outs.append(o.reshape(R, HP1, CW).reshape(BC, L, HP1, CW))
    return np.concatenate(outs, axis=0), res


def kernel(x0: np.ndarray, controls: np.ndarray, deg: np.ndarray) -> np.ndarray:
    out, _ = _run(x0, controls, deg)
    return out


if __name__ == "__main__":
    rng = np.random.default_rng(0)
    x0 = rng.standard_normal((B, 12)).astype(np.float32)
    controls = rng.standard_normal((B, L, H, 3)).astype(np.float32)
    deg = rng.random((B, 5)).astype(np.float32)
    out = kernel(x0, controls, deg)
    print("out", out.shape, out.dtype)


# revision 22
# speedup vs baseline: 1.5744x; 1.0320x over previous
"""Degraded bicycle rollout kernel for Trainium2 (8 NeuronCores, data-parallel on batch).

v3 (validated vs reference in numpy, worst rel err ~5e-4 vs 2e-2 tol):
  - Serial 80-step speed recurrence replaced by 3 parallel scans using
        relu-prefix(a)_t = A_t - runmin(min(A,0))_t,  A = s0 + cumsum(a)
    (drops the 1e-6 epsilon inside sqrt; error <= ~1e-3 absolute, non-compounding).
  - Per-partition scalars (deg-derived scales, s0, x0 columns) precomputed on host.
  - 4 chunks x 8 rollouts; yaw/vx/vy written strided directly into interleaved
    staging and re-read strided by the scans/diffs (no dense copies).
  - psi+beta fused via affine_then_add; per-chunk work split into two stages
    and software-pipelined; activation-table order: sigmoid-set (q0) | trig
    (chunk0) | sigmoid-set (q1-3) | trig (chunks 1-3)  -> 4 table loads.

Layout per core: 4096 rollouts = 128 partitions x 32; partition p holds
rollouts p*32..p*32+31, all from batch bc = p//2 (per-batch scalars are
per-partition scalars). Free dim rollout-major: f = n*80 + t.
"""

import sys

sys.path.insert(0, "/opt/trn_rl_repo")

import numpy as np

B, L, H = 512, 64, 80
NCORES = 8
BC = B // NCORES          # 64 batches per core
R = BC * L                # 4096 rollouts per core
P = 128
NPT = R // P              # 32 rollouts per partition
F = NPT * H               # 2560 elements per partition
HP1 = H + 1
CW = 12                   # output channels
SW = HP1 * CW             # 972 staging words per rollout
NCH = 4                   # chunks
NG = NPT // NCH           # 8 rollouts per partition per chunk
CF = NG * H               # 640 free elems per chunk
CHW = NG * SW             # 7776 staging words per partition per chunk
QW = NG * H * 3           # 1920 ctrl words per partition per quarter
DT = 0.1
WB = 2.8
PI = float(np.pi)
BIG = 1e30

_BUILT = None


def _build_kernel():
    import concourse.bass as bass
    import concourse.bacc as bacc
    import concourse.mybir as mybir
    from concourse.tile import TileContext
    from concourse.tile_rust import add_dep_helper

    f32 = mybir.dt.float32
    A = mybir.AluOpType
    AF = mybir.ActivationFunctionType

    nc = bacc.Bacc(None, target_bir_lowering=False)
    ctrl_d = nc.declare_dram_parameter("ctrl", [P, NPT * H * 3], f32, isOutput=False)
    x0_d = nc.declare_dram_parameter("x0p", [P, 12], f32, isOutput=False)
    scl_d = nc.declare_dram_parameter("sclp", [P, 16], f32, isOutput=False)
    out_d = nc.declare_dram_parameter("out", [P, NPT * SW], f32, isOutput=True)

    with TileContext(nc) as tc:
        v = nc.vector
        sc = nc.scalar
        gp = nc.gpsimd
        sy = nc.sync

        with tc.tile_pool(name="pers", bufs=1) as pp, \
             tc.tile_pool(name="ctrlp", bufs=2) as cp, \
             tc.tile_pool(name="wk1", bufs=1) as w1, \
             tc.tile_pool(name="wk2", bufs=2) as w2, \
             tc.tile_pool(name="stgp", bufs=2) as sp, \
             tc.tile_pool(name="psp", bufs=2, space="PSUM") as ps:

            x0s = pp.tile([P, 12], f32, tag="x0s")
            scl = pp.tile([P, 16], f32, tag="scl")
            steer = scl[:, 0:1]
            brake = scl[:, 1:2]
            thr = scl[:, 2:3]
            b65 = scl[:, 3:4]
            t28 = scl[:, 4:5]
            lo75 = scl[:, 5:6]
            invf981 = scl[:, 6:7]
            s0v = scl[:, 7:8]
            psi0 = scl[:, 8:9]
            px0 = scl[:, 9:10]
            py0 = scl[:, 10:11]
            vx010 = scl[:, 11:12]
            vy010 = scl[:, 12:13]
            hpiv = scl[:, 13:14]
            psi0dt = scl[:, 14:15]

            # scan masks (same pattern for every chunk)
            maskc = pp.tile([P, CF], f32, tag="maskc")
            v.memset(maskc[:], 1.0)
            mc3 = maskc[:].rearrange("p (n h) -> p n h", n=NG)
            v.memset(mc3[:, :, 0:1], 0.0)
            bigm = pp.tile([P, CF], f32, tag="bigm")
            v.memset(bigm[:], 0.0)
            bg3 = bigm[:].rearrange("p (n h) -> p n h", n=NG)
            v.memset(bg3[:, :, 0:1], BIG)

            # persistent control transforms (sg1/sg2 interleaved pairwise)
            th = pp.tile([P, F], f32, tag="th")
            sg = pp.tile([P, 2 * F], f32, tag="sg")
            accDT = pp.tile([P, F], f32, tag="accDT")
            th3 = th[:].rearrange("p (n h) -> p n h", n=NPT)
            sg4 = sg[:].rearrange("p (n h c) -> p n h c", n=NPT, c=2)

            acts = {}

            def phaseA_act(q):
                n0 = q * NG
                nsl = slice(n0, n0 + NG)
                ctrl = cp.tile([P, QW], f32, tag="ctrlq")
                sy.dma_start(out=ctrl[:], in_=ctrl_d[:, q * QW:(q + 1) * QW])
                c4 = ctrl[:].rearrange("p (n h c) -> p n h c", n=NG, h=H)
                sc.activation(th3[:, nsl, :], c4[:, :, :, 0], AF.Tanh)
                i = sc.activation(sg4[:, nsl, :, :], c4[:, :, :, 1:3], AF.Sigmoid)
                acts[f"sg_{q}"] = i

            def phaseA_acc(q):
                n0 = q * NG
                nsl = slice(n0, n0 + NG)
                csl = slice(q * CF, (q + 1) * CF)
                fb65 = w1.tile([P, CF], f32, tag="w_a")
                v.tensor_scalar(fb65[:], sg4[:, nsl, :, 0], b65, None, A.mult)
                t3 = w1.tile([P, CF], f32, tag="w_b")
                v.scalar_tensor_tensor(t3[:], sg4[:, nsl, :, 1], t28, fb65[:],
                                       A.mult, A.subtract)
                v.tensor_scalar(accDT[:, csl], t3[:], 0.3, lo75, A.min, A.max)

            def phaseA(q):
                phaseA_act(q)
                phaseA_acc(q)

            stgs = {}
            carry = {}

            def stage1(g):
                n0 = g * NG
                nsl = slice(n0, n0 + NG)
                csl = slice(g * CF, (g + 1) * CF)

                stg = sp.tile([P, CHW], f32, tag="stg")
                stgs[g] = stg
                s4 = stg[:].rearrange("p (n t c) -> p n t c", n=NG, t=HP1)

                # t=0 slice: full x0 row broadcast over rollouts
                x0b = x0s[:, None, 0:12].broadcast_to([P, NG, 12])
                v.tensor_scalar(s4[:, :, 0, :], x0b, 1.0, None, A.mult)

                # ch9 delta = steer*tanh(u0); dc = clip(delta, +-0.75)
                v.tensor_scalar(s4[:, :, 1:, 9], th3[:, nsl, :], steer, None,
                                A.mult)
                dc = w2.tile([P, CF], f32, tag="dc")
                dc3 = dc[:].rearrange("p (n h) -> p n h", n=NG)
                v.tensor_scalar(dc3, s4[:, :, 1:, 9], 0.75, -0.75, A.min, A.max)

                # trig of steering
                sin_d = w2.tile([P, CF], f32, tag="sin_d")
                i_sin = sc.activation(sin_d[:], dc[:], AF.Sin)
                acts[f"sin_{g}"] = i_sin
                cos_d = w2.tile([P, CF], f32, tag="cos_d")
                sc.activation(cos_d[:], dc[:], AF.Sin, bias=hpiv)
                # ch10/ch11: brake*sg1, thr*sg2
                sc.activation(s4[:, :, 1:, 10], sg4[:, nsl, :, 0], AF.Identity,
                              scale=brake)
                sc.activation(s4[:, :, 1:, 11], sg4[:, nsl, :, 1], AF.Identity,
                              scale=thr)

                # speed2 scans: C=cumsum(accDT); s2 = (C+s0) - runmin(min(C+s0,0))
                C = w1.tile([P, CF], f32, tag="C")
                v.tensor_tensor_scan(C[:], maskc[:], accDT[:, csl], 0.0,
                                     A.mult, A.add)
                Bm = w1.tile([P, CF], f32, tag="w_a")
                v.tensor_scalar(Bm[:], C[:], s0v, 0.0, A.add, A.min)
                mrun = w1.tile([P, CF], f32, tag="w_b")
                v.tensor_tensor_scan(mrun[:], bigm[:], Bm[:], 0.0, A.add, A.min)
                s2 = w2.tile([P, CF], f32, tag="s2")
                v.scalar_tensor_tensor(s2[:], C[:], s0v, mrun[:], A.add,
                                       A.subtract)

                rc = w1.tile([P, CF], f32, tag="rc")
                v.reciprocal_approx_fast(rc[:], cos_d[:])
                tan = w2.tile([P, CF], f32, tag="tan")
                gp.tensor_tensor(tan[:], sin_d[:], rc[:], A.mult)
                tan3 = tan[:].rearrange("p (n h) -> p n h", n=NG)
                sc.activation(s4[:, :, 1:, 8], tan3, AF.Arctan, scale=0.45)
                mchi = w1.tile([P, CF], f32, tag="mchi")
                gp.tensor_scalar(mchi[:], s2[:], 2.0, invf981, A.max, A.mult)
                imc = w1.tile([P, CF], f32, tag="imc")
                v.reciprocal_approx_fast(imc[:], mchi[:])

                # yaw_rate into ch5 (strided), psi cumsum reads it back strided
                rawc = w1.tile([P, CF], f32, tag="w_a")
                v.scalar_tensor_tensor(rawc[:], s2[:], 1.0 / WB, tan[:],
                                       A.mult, A.mult)
                clpc = w1.tile([P, CF], f32, tag="w_b")
                v.tensor_scalar(clpc[:], rawc[:], 1.0, -1.0, A.min, A.max)
                yawc = w1.tile([P, CF], f32, tag="yawc")
                v.scalar_tensor_tensor(yawc[:], imc[:], 0.15, clpc[:],
                                       A.max, A.mult)
                yw3 = yawc[:].rearrange("p (n h) -> p n h", n=NG)
                sc.activation(s4[:, :, 1:, 5], yw3, AF.Identity)
                # inject psi0/DT at each rollout start so the cumsum carries psi0
                v.tensor_scalar(yw3[:, :, 0], yw3[:, :, 0], psi0dt, None, A.add)
                Fp = w1.tile([P, CF], f32, tag="Fp")
                Fp3 = Fp[:].rearrange("p (n h) -> p n h", n=NG)
                v.tensor_tensor_scan(Fp[:], maskc[:], yawc[:], 0.0, A.mult, A.add)
                sc.activation(s4[:, :, 1:, 2], Fp3, AF.Identity, scale=DT)

                # arg = Fp*DT + beta in one DVE op; wrap for sin/cos
                argc = w2.tile([P, CF], f32, tag="argc")
                v.affine_then_add(argc[:].rearrange("p (n h) -> p n h", n=NG),
                                  Fp3, s4[:, :, 1:, 8], DT, 0.0)
                argw = ps.tile([P, CF], f32, tag="argw")
                v.add_range_wrap(argw[:], argc[:], 0.0, PI, 2 * PI)
                cwv = ps.tile([P, CF], f32, tag="cwv")
                v.add_range_wrap(cwv[:], argc[:], PI / 2, PI, 2 * PI)
                carry[g] = (s2, argw, cwv)

            def stage2(g):
                n0 = g * NG
                nsl = slice(n0, n0 + NG)
                stg = stgs.pop(g)
                s4 = stg[:].rearrange("p (n t c) -> p n t c", n=NG, t=HP1)
                s2, argw, cwv = carry.pop(g)

                sinA = w1.tile([P, CF], f32, tag="sinA")
                i_sinA = sc.activation(sinA[:], argw[:], AF.Sin)
                acts[f"sinA_{g}"] = i_sinA
                cosA = w1.tile([P, CF], f32, tag="cosA")
                i_cosA = sc.activation(cosA[:], cwv[:], AF.Sin)
                acts[f"cosA_{g}"] = i_cosA

                # velocities (dense; copies into ch3/ch4 on ACT)
                vx2 = w1.tile([P, CF], f32, tag="vx2")
                gp.tensor_tensor(vx2[:], s2[:], cosA[:], A.mult)
                vy2 = w1.tile([P, CF], f32, tag="vy2")
                gp.tensor_tensor(vy2[:], s2[:], sinA[:], A.mult)
                vx3 = vx2[:].rearrange("p (n h) -> p n h", n=NG)
                vy3 = vy2[:].rearrange("p (n h) -> p n h", n=NG)
                sc.activation(s4[:, :, 1:, 3], vx3, AF.Identity)
                sc.activation(s4[:, :, 1:, 4], vy3, AF.Identity)

                # px/py cumsums
                Fx = w1.tile([P, CF], f32, tag="Fx")
                Fx3 = Fx[:].rearrange("p (n h) -> p n h", n=NG)
                v.tensor_tensor_scan(Fx[:], maskc[:], vx2[:], 0.0,
                                     A.mult, A.add)
                sc.activation(s4[:, :, 1:, 0], Fx3, AF.Identity, bias=px0,
                              scale=DT)
                Fy = w1.tile([P, CF], f32, tag="Fy")
                Fy3 = Fy[:].rearrange("p (n h) -> p n h", n=NG)
                v.tensor_tensor_scan(Fy[:], maskc[:], vy2[:], 0.0,
                                     A.mult, A.add)
                sc.activation(s4[:, :, 1:, 1], Fy3, AF.Identity, bias=py0,
                              scale=DT)

                # ax/ay: diff into ch6/ch7, then in-place *10
                gp.tensor_tensor(s4[:, :, 2:, 6], vx3[:, :, 1:],
                                 vx3[:, :, :H - 1], A.subtract)
                gp.tensor_scalar(s4[:, :, 2:, 6], s4[:, :, 2:, 6], 10.0, None,
                                 A.mult)
                v.tensor_scalar(s4[:, :, 1, 6], vx3[:, :, 0], 10.0, vx010,
                                A.mult, A.subtract)
                gp.tensor_tensor(s4[:, :, 2:, 7], vy3[:, :, 1:],
                                 vy3[:, :, :H - 1], A.subtract)
                gp.tensor_scalar(s4[:, :, 2:, 7], s4[:, :, 2:, 7], 10.0, None,
                                 A.mult)
                v.tensor_scalar(s4[:, :, 1, 7], vy3[:, :, 0], 10.0, vy010,
                                A.mult, A.subtract)

                sy.dma_start(out=out_d[:, g * CHW:(g + 1) * CHW], in_=stg[:])

            # emission: ctrl q0 first so phase A starts ASAP, then small DMAs
            phaseA_act(0)
            sy.dma_start(out=x0s[:], in_=x0_d[:])
            sy.dma_start(out=scl[:], in_=scl_d[:])
            phaseA_acc(0)
            stage1(0)
            stage2(0)
            phaseA(1)
            phaseA(2)
            phaseA(3)
            stage1(1)
            stage2(1)
            stage1(2)
            stage2(2)
            stage1(3)
            stage2(3)

            # activation-table grouping: sig(q0) | trig(c0) | sig(q1-3) | trig
            add_dep_helper(acts["sin_0"].ins, acts["sg_0"].ins,
                           reason="table: q0 sigmoid before c0 trig")
            for q in (1, 2, 3):
                add_dep_helper(acts[f"sg_{q}"].ins, acts["cosA_0"].ins,
                               reason="table: c0 trig before q1-3 sigmoid")
            add_dep_helper(acts["sin_1"].ins, acts["sg_3"].ins,
                           reason="table: q1-3 sigmoid before c1+ trig")

    nc.compile()
    return nc


def _get_built():
    global _BUILT
    if _BUILT is None:
        _BUILT = _build_kernel()
    return _BUILT


def _make_scalars(x0, deg):
    """Per-batch scalar table [B, 16] (host precompute of deg-derived scales)."""
    steer = np.maximum(deg[:, 0], 0.05)
    brake = np.maximum(deg[:, 1], 0.05)
    thr = np.maximum(deg[:, 2], 0.05)
    fric = np.maximum(deg[:, 4], 0.1)
    vx0 = x0[:, 3]
    vy0 = x0[:, 4]
    scl = np.zeros((B, 16), np.float32)
    scl[:, 0] = steer
    scl[:, 1] = brake
    scl[:, 2] = thr
    scl[:, 3] = 0.65 * brake
    scl[:, 4] = 0.28 * thr
    scl[:, 5] = -0.75 * fric
    scl[:, 6] = 1.0 / (9.81 * fric)
    scl[:, 7] = np.sqrt(vx0 * vx0 + vy0 * vy0 + 1e-6)
    scl[:, 8] = x0[:, 2]
    scl[:, 9] = x0[:, 0]
    scl[:, 10] = x0[:, 1]
    scl[:, 11] = 10.0 * vx0
    scl[:, 12] = 10.0 * vy0
    scl[:, 13] = PI / 2
    scl[:, 14] = 10.0 * x0[:, 2]
    return scl


def _run(x0, controls, deg, trace=False):
    from concourse.bass_utils import run_bass_kernel_spmd

    x0 = np.ascontiguousarray(x0, dtype=np.float32)
    controls = np.ascontiguousarray(controls, dtype=np.float32)
    deg = np.ascontiguousarray(deg, dtype=np.float32)
    scl = _make_scalars(x0, deg)

    nc = _get_built()
    rep = P // BC
    in_maps = []
    for c in range(NCORES):
        sl = slice(c * BC, (c + 1) * BC)
        ctrl_c = controls[sl].reshape(R, H * 3).reshape(P, NPT * H * 3)
        in_maps.append({
            "ctrl": np.ascontiguousarray(ctrl_c),
            "x0p": np.ascontiguousarray(np.repeat(x0[sl], rep, axis=0)),
            "sclp": np.ascontiguousarray(np.repeat(scl[sl], rep, axis=0)),
        })

    res = run_bass_kernel_spmd(nc, in_maps, list(range(NCORES)), trace=trace)
    outs = []
    for c in range(NCORES):
        o = np.asarray(res.results[c]["out"])
        outs.append(o.reshape(R, HP1, CW).reshape(BC, L, HP1, CW))
    return np.concatenate(outs, axis=0), res


def kernel(x0: np.ndarray, controls: np.ndarray, deg: np.ndarray) -> np.ndarray:
    out, _ = _run(x0, controls, deg)
    return out


if __name__ == "__main__":
    rng = np.random.default_rng(0)
    x0 = rng.standard_normal((B, 12)).astype(np.float32)
    controls = rng.standard_normal((B, L, H, 3)).astype(np.float32)
    deg = rng.random((B, 5)).astype(np.float32)
    out = kernel(x0, controls, deg)
    print("out", out.shape, out.dtype)


# revision 32
# speedup vs baseline: 1.7093x; 1.0857x over previous
"""Degraded bicycle rollout kernel for Trainium2 (8 NeuronCores, data-parallel on batch).

v3 (validated vs reference in numpy, worst rel err ~5e-4 vs 2e-2 tol):
  - Serial 80-step speed recurrence replaced by 3 parallel scans using
        relu-prefix(a)_t = A_t - runmin(min(A,0))_t,  A = s0 + cumsum(a)
    (drops the 1e-6 epsilon inside sqrt; error <= ~1e-3 absolute, non-compounding).
  - Per-partition scalars (deg-derived scales, s0, x0 columns) precomputed on host.
  - 4 chunks x 8 rollouts; yaw/vx/vy written strided directly into interleaved
    staging and re-read strided by the scans/diffs (no dense copies).
  - psi+beta fused via affine_then_add; per-chunk work split into two stages
    and software-pipelined; activation-table order: sigmoid-set (q0) | trig
    (chunk0) | sigmoid-set (q1-3) | trig (chunks 1-3)  -> 4 table loads.

Layout per core: 4096 rollouts = 128 partitions x 32; partition p holds
rollouts p*32..p*32+31, all from batch bc = p//2 (per-batch scalars are
per-partition scalars). Free dim rollout-major: f = n*80 + t.
"""

import sys

sys.path.insert(0, "/opt/trn_rl_repo")

import numpy as np

B, L, H = 512, 64, 80
NCORES = 8
BC = B // NCORES          # 64 batches per core
R = BC * L                # 4096 rollouts per core
P = 128
NPT = R // P              # 32 rollouts per partition
F = NPT * H               # 2560 elements per partition
HP1 = H + 1
CW = 12                   # output channels
SW = HP1 * CW             # 972 staging words per rollout
NCH = 4                   # chunks
NG = NPT // NCH           # 8 rollouts per partition per chunk
CF = NG * H               # 640 free elems per chunk
CHW = NG * SW             # 7776 staging words per partition per chunk
QW = NG * H * 3           # 1920 ctrl words per partition per quarter
DT = 0.1
WB = 2.8
PI = float(np.pi)
BIG = 1e30

_BUILT = None


def _build_kernel():
    import concourse.bass as bass
    import concourse.bacc as bacc
    import concourse.mybir as mybir
    from concourse.tile import TileContext
    from concourse.tile_rust import add_dep_helper

    f32 = mybir.dt.float32
    A = mybir.AluOpType
    AF = mybir.ActivationFunctionType

    nc = bacc.Bacc(None, target_bir_lowering=False)
    ctrl_d = nc.declare_dram_parameter("ctrl", [P, NPT * H * 3], f32, isOutput=False)
    x0_d = nc.declare_dram_parameter("x0p", [P, 12], f32, isOutput=False)
    scl_d = nc.declare_dram_parameter("sclp", [P, 16], f32, isOutput=False)
    out_d = nc.declare_dram_parameter("out", [P, NPT * SW], f32, isOutput=True)

    with TileContext(nc) as tc:
        v = nc.vector
        sc = nc.scalar
        gp = nc.gpsimd
        sy = nc.sync

        with tc.tile_pool(name="pers", bufs=1) as pp, \
             tc.tile_pool(name="ctrlp", bufs=4) as cp, \
             tc.tile_pool(name="wk1", bufs=1) as w1, \
             tc.tile_pool(name="wk2", bufs=2) as w2, \
             tc.tile_pool(name="stgp", bufs=2) as sp, \
             tc.tile_pool(name="psp", bufs=2, space="PSUM") as ps:

            x0s = pp.tile([P, 12], f32, tag="x0s")
            scl = pp.tile([P, 16], f32, tag="scl")
            steer = scl[:, 0:1]
            brake = scl[:, 1:2]
            thr = scl[:, 2:3]
            b65 = scl[:, 3:4]
            t28 = scl[:, 4:5]
            lo75 = scl[:, 5:6]
            invf981 = scl[:, 6:7]
            s0v = scl[:, 7:8]
            psi0 = scl[:, 8:9]
            px0 = scl[:, 9:10]
            py0 = scl[:, 10:11]
            vx010 = scl[:, 11:12]
            vy010 = scl[:, 12:13]
            hpiv = scl[:, 13:14]
            psi0dt = scl[:, 14:15]

            # scan masks (same pattern for every chunk)
            maskc = pp.tile([P, CF], f32, tag="maskc")
            v.memset(maskc[:], 1.0)
            mc3 = maskc[:].rearrange("p (n h) -> p n h", n=NG)
            v.memset(mc3[:, :, 0:1], 0.0)
            bigm = pp.tile([P, CF], f32, tag="bigm")
            v.memset(bigm[:], 0.0)
            bg3 = bigm[:].rearrange("p (n h) -> p n h", n=NG)
            v.memset(bg3[:, :, 0:1], BIG)

            # persistent control transforms (sg1/sg2 interleaved pairwise)
            th = pp.tile([P, F], f32, tag="th")
            sg = pp.tile([P, 2 * F], f32, tag="sg")
            accDT = pp.tile([P, F], f32, tag="accDT")
            th3 = th[:].rearrange("p (n h) -> p n h", n=NPT)
            sg4 = sg[:].rearrange("p (n h c) -> p n h c", n=NPT, c=2)

            acts = {}

            ctrls = {}

            def ctrl_dma(q):
                ctrl = cp.tile([P, QW], f32, tag="ctrlq")
                sy.dma_start(out=ctrl[:], in_=ctrl_d[:, q * QW:(q + 1) * QW])
                ctrls[q] = ctrl

            def phaseA_act(q):
                n0 = q * NG
                nsl = slice(n0, n0 + NG)
                c4 = ctrls.pop(q)[:].rearrange("p (n h c) -> p n h c", n=NG, h=H)
                sc.activation(th3[:, nsl, :], c4[:, :, :, 0], AF.Tanh)
                i = sc.activation(sg4[:, nsl, :, :], c4[:, :, :, 1:3], AF.Sigmoid)
                acts[f"sg_{q}"] = i

            def phaseA_acc(q):
                n0 = q * NG
                nsl = slice(n0, n0 + NG)
                csl = slice(q * CF, (q + 1) * CF)
                fb65 = w1.tile([P, CF], f32, tag="pa_a")
                v.tensor_scalar(fb65[:], sg4[:, nsl, :, 0], b65, None, A.mult)
                t3 = w1.tile([P, CF], f32, tag="pa_b")
                v.scalar_tensor_tensor(t3[:], sg4[:, nsl, :, 1], t28, fb65[:],
                                       A.mult, A.subtract)
                v.tensor_scalar(accDT[:, csl], t3[:], 0.3, lo75, A.min, A.max)

            def phaseA(q):
                phaseA_act(q)
                phaseA_acc(q)

            stgs = {}
            carry = {}

            def stage1(g):
                n0 = g * NG
                nsl = slice(n0, n0 + NG)
                csl = slice(g * CF, (g + 1) * CF)

                stg = sp.tile([P, CHW], f32, tag="stg")
                stgs[g] = stg
                s4 = stg[:].rearrange("p (n t c) -> p n t c", n=NG, t=HP1)

                # t=0 slice: full x0 row broadcast over rollouts
                x0b = x0s[:, None, 0:12].broadcast_to([P, NG, 12])
                v.tensor_scalar(s4[:, :, 0, :], x0b, 1.0, None, A.mult)

                # ch9 delta = steer*tanh(u0); dc = clip(delta, +-0.75)
                v.tensor_scalar(s4[:, :, 1:, 9], th3[:, nsl, :], steer, None,
                                A.mult)
                dc = w2.tile([P, CF], f32, tag="dc")
                dc3 = dc[:].rearrange("p (n h) -> p n h", n=NG)
                gp.tensor_scalar(dc3, s4[:, :, 1:, 9], 0.75, -0.75, A.min, A.max)

                # trig of steering
                sin_d = w2.tile([P, CF], f32, tag="sin_d")
                i_sin = sc.activation(sin_d[:], dc[:], AF.Sin)
                acts[f"sin_{g}"] = i_sin
                cos_d = w2.tile([P, CF], f32, tag="cos_d")
                sc.activation(cos_d[:], dc[:], AF.Sin, bias=hpiv)
                # ch10/ch11: brake*sg1, thr*sg2
                sc.activation(s4[:, :, 1:, 10], sg4[:, nsl, :, 0], AF.Identity,
                              scale=brake)
                sc.activation(s4[:, :, 1:, 11], sg4[:, nsl, :, 1], AF.Identity,
                              scale=thr)

                # speed2 scans: C=cumsum(accDT); s2 = (C+s0) - runmin(min(C+s0,0))
                C = w1.tile([P, CF], f32, tag="C")
                v.tensor_tensor_scan(C[:], maskc[:], accDT[:, csl], 0.0,
                                     A.mult, A.add)
                Bm = w1.tile([P, CF], f32, tag="w_a")
                gp.tensor_scalar(Bm[:], C[:], s0v, 0.0, A.add, A.min)
                mrun = w1.tile([P, CF], f32, tag="w_b")
                v.tensor_tensor_scan(mrun[:], bigm[:], Bm[:], 0.0, A.add, A.min)
                s2 = w2.tile([P, CF], f32, tag="s2")
                v.scalar_tensor_tensor(s2[:], C[:], s0v, mrun[:], A.add,
                                       A.subtract)

                rc = w1.tile([P, CF], f32, tag="rc")
                v.reciprocal_approx_fast(rc[:], cos_d[:])
                tan = w2.tile([P, CF], f32, tag="tan")
                gp.tensor_tensor(tan[:], sin_d[:], rc[:], A.mult)
                tan3 = tan[:].rearrange("p (n h) -> p n h", n=NG)
                sc.activation(s4[:, :, 1:, 8], tan3, AF.Arctan, scale=0.45)
                mchi = w1.tile([P, CF], f32, tag="mchi")
                gp.tensor_scalar(mchi[:], s2[:], 2.0, invf981, A.max, A.mult)
                imc = w1.tile([P, CF], f32, tag="imc")
                v.reciprocal_approx_fast(imc[:], mchi[:])

                # yaw_rate into ch5 (strided), psi cumsum reads it back strided
                rawc = w1.tile([P, CF], f32, tag="w_a")
                v.scalar_tensor_tensor(rawc[:], s2[:], 1.0 / WB, tan[:],
                                       A.mult, A.mult)
                clpc = w1.tile([P, CF], f32, tag="w_b")
                gp.tensor_scalar(clpc[:], rawc[:], 1.0, -1.0, A.min, A.max)
                yawc = w1.tile([P, CF], f32, tag="yawc")
                v.scalar_tensor_tensor(yawc[:], imc[:], 0.15, clpc[:],
                                       A.max, A.mult)
                yw3 = yawc[:].rearrange("p (n h) -> p n h", n=NG)
                sc.activation(s4[:, :, 1:, 5], yw3, AF.Identity)
                # inject psi0/DT at each rollout start so the cumsum carries psi0
                v.tensor_scalar(yw3[:, :, 0], yw3[:, :, 0], psi0dt, None, A.add)
                Fp = w1.tile([P, CF], f32, tag="Fp")
                Fp3 = Fp[:].rearrange("p (n h) -> p n h", n=NG)
                v.tensor_tensor_scan(Fp[:], maskc[:], yawc[:], 0.0, A.mult, A.add)
                sc.activation(s4[:, :, 1:, 2], Fp3, AF.Identity, scale=DT)

                # arg = Fp*DT + beta in one DVE op; wrap for sin/cos
                argc = w2.tile([P, CF], f32, tag="argc")
                v.affine_then_add(argc[:].rearrange("p (n h) -> p n h", n=NG),
                                  Fp3, s4[:, :, 1:, 8], DT, 0.0)
                argw = ps.tile([P, CF], f32, tag="argw")
                v.add_range_wrap(argw[:], argc[:], 0.0, PI, 2 * PI)
                cwv = ps.tile([P, CF], f32, tag="cwv")
                v.add_range_wrap(cwv[:], argc[:], PI / 2, PI, 2 * PI)
                carry[g] = (s2, argw, cwv)

            def stage2(g):
                n0 = g * NG
                nsl = slice(n0, n0 + NG)
                stg = stgs.pop(g)
                s4 = stg[:].rearrange("p (n t c) -> p n t c", n=NG, t=HP1)
                s2, argw, cwv = carry.pop(g)

                sinA = w1.tile([P, CF], f32, tag="sinA")
                i_sinA = sc.activation(sinA[:], argw[:], AF.Sin)
                acts[f"sinA_{g}"] = i_sinA
                cosA = w1.tile([P, CF], f32, tag="cosA")
                i_cosA = sc.activation(cosA[:], cwv[:], AF.Sin)
                acts[f"cosA_{g}"] = i_cosA

                # velocities scaled by 10 (w = 10*s2*cos); ch3 = 0.1*w
                w10x = w1.tile([P, CF], f32, tag="w10x")
                v.scalar_tensor_tensor(w10x[:], s2[:], 10.0, cosA[:],
                                       A.mult, A.mult)
                w10y = w1.tile([P, CF], f32, tag="w10y")
                v.scalar_tensor_tensor(w10y[:], s2[:], 10.0, sinA[:],
                                       A.mult, A.mult)
                wx3 = w10x[:].rearrange("p (n h) -> p n h", n=NG)
                wy3 = w10y[:].rearrange("p (n h) -> p n h", n=NG)
                sc.activation(s4[:, :, 1:, 3], wx3, AF.Identity, scale=0.1)
                sc.activation(s4[:, :, 1:, 4], wy3, AF.Identity, scale=0.1)

                # px/py cumsums (of 10*v, so scale DT/10)
                Fx = w1.tile([P, CF], f32, tag="Fx")
                Fx3 = Fx[:].rearrange("p (n h) -> p n h", n=NG)
                v.tensor_tensor_scan(Fx[:], maskc[:], w10x[:], 0.0,
                                     A.mult, A.add)
                sc.activation(s4[:, :, 1:, 0], Fx3, AF.Identity, bias=px0,
                              scale=DT / 10.0)
                Fy = w1.tile([P, CF], f32, tag="Fy")
                Fy3 = Fy[:].rearrange("p (n h) -> p n h", n=NG)
                v.tensor_tensor_scan(Fy[:], maskc[:], w10y[:], 0.0,
                                     A.mult, A.add)
                sc.activation(s4[:, :, 1:, 1], Fy3, AF.Identity, bias=py0,
                              scale=DT / 10.0)

                # ax/ay: diffs of w10 are already scaled
                gp.tensor_tensor(s4[:, :, 2:, 6], wx3[:, :, 1:],
                                 wx3[:, :, :H - 1], A.subtract)
                v.tensor_scalar(s4[:, :, 1, 6], wx3[:, :, 0], 1.0, vx010,
                                A.mult, A.subtract)
                gp.tensor_tensor(s4[:, :, 2:, 7], wy3[:, :, 1:],
                                 wy3[:, :, :H - 1], A.subtract)
                v.tensor_scalar(s4[:, :, 1, 7], wy3[:, :, 0], 1.0, vy010,
                                A.mult, A.subtract)

                sy.dma_start(out=out_d[:, g * CHW:(g + 1) * CHW], in_=stg[:])

            # emission: ALL input DMAs first so nothing queues behind outputs
            sy.dma_start(out=x0s[:], in_=x0_d[:])
            sy.dma_start(out=scl[:], in_=scl_d[:])
            for q in range(NCH):
                ctrl_dma(q)
            phaseA(0)
            stage1(0)
            stage2(0)
            phaseA(1)
            phaseA(2)
            phaseA(3)
            stage1(1)
            stage2(1)
            stage1(2)
            stage2(2)
            stage1(3)
            stage2(3)

            # activation-table grouping: sig(q0) | trig(c0) | sig(q1-3) | trig
            add_dep_helper(acts["sin_0"].ins, acts["sg_0"].ins,
                           reason="table: q0 sigmoid before c0 trig")
            for q in (1, 2, 3):
                add_dep_helper(acts[f"sg_{q}"].ins, acts["cosA_0"].ins,
                               reason="table: c0 trig before q1-3 sigmoid")
            add_dep_helper(acts["sin_1"].ins, acts["sg_3"].ins,
                           reason="table: q1-3 sigmoid before c1+ trig")

    nc.compile()
    return nc


def _get_built():
    global _BUILT
    if _BUILT is None:
        _BUILT = _build_kernel()
    return _BUILT


def _make_scalars(x0, deg):
    """Per-batch scalar table [B, 16] (host precompute of deg-derived scales)."""
    steer = np.maximum(deg[:, 0], 0.05)
    brake = np.maximum(deg[:, 1], 0.05)
    thr = np.maximum(deg[:, 2], 0.05)
    fric = np.maximum(deg[:, 4], 0.1)
    vx0 = x0[:, 3]
    vy0 = x0[:, 4]
    scl = np.zeros((B, 16), np.float32)
    scl[:, 0] = steer
    scl[:, 1] = brake
    scl[:, 2] = thr
    scl[:, 3] = 0.65 * brake
    scl[:, 4] = 0.28 * thr
    scl[:, 5] = -0.75 * fric
    scl[:, 6] = 1.0 / (9.81 * fric)
    scl[:, 7] = np.sqrt(vx0 * vx0 + vy0 * vy0 + 1e-6)
    scl[:, 8] = x0[:, 2]
    scl[:, 9] = x0[:, 0]
    scl[:, 10] = x0[:, 1]
    scl[:, 11] = 10.0 * vx0
    scl[:, 12] = 10.0 * vy0
    scl[:, 13] = PI / 2
    scl[:, 14] = 10.0 * x0[:, 2]
    return scl


def _run(x0, controls, deg, trace=False):
    from concourse.bass_utils import run_bass_kernel_spmd

    x0 = np.ascontiguousarray(x0, dtype=np.float32)
    controls = np.ascontiguousarray(controls, dtype=np.float32)
    deg = np.ascontiguousarray(deg, dtype=np.float32)
    scl = _make_scalars(x0, deg)

    nc = _get_built()
    rep = P // BC
    in_maps = []
    for c in range(NCORES):
        sl = slice(c * BC, (c + 1) * BC)
        ctrl_c = controls[sl].reshape(R, H * 3).reshape(P, NPT * H * 3)
        in_maps.append({
            "ctrl": np.ascontiguousarray(ctrl_c),
            "x0p": np.ascontiguousarray(np.repeat(x0[sl], rep, axis=0)),
            "sclp": np.ascontiguousarray(np.repeat(scl[sl], rep, axis=0)),
        })

    res = run_bass_kernel_spmd(nc, in_maps, list(range(NCORES)), trace=trace)
    outs = []
    for c in range(NCORES):
        o = np.asarray(res.results[c]["out"])
        outs.append(o.reshape(R, HP1, CW).reshape(BC, L, HP1, CW))
    return np.concatenate(outs, axis=0), res


def kernel(x0: np.ndarray, controls: np.ndarray, deg: np.ndarray) -> np.ndarray:
    out, _ = _run(x0, controls, deg)
    return out


if __name__ == "__main__":
    rng = np.random.default_rng(0)
    x0 = rng.standard_normal((B, 12)).astype(np.float32)
    controls = rng.standard_normal((B, L, H, 3)).astype(np.float32)
    deg = rng.random((B, 5)).astype(np.float32)
    out = kernel(x0, controls, deg)
    print("out", out.shape, out.dtype)
